# revision 8
# baseline (speedup 1.0000x reference)
"""Masked multi-head self-attention for Trainium2, SPMD over 8 NeuronCores.

Sharding: core c handles batch c//2, query-half c%2 (1024 of 2048 query rows).
The same Bass program runs on every core; odd cores get their inputs rotated
along the key axis so that "my" query rows are always tokens [0, 1024)
(attention sums are invariant to a consistent permutation of the key axis).

Host supplies x^T (features-major q) and (1-mask)^T in bf16, so the device
does no transposes. Per-core dataflow:
  Q^T/K^T (head-pair packed, fp32r) and V (token-major bf16, with a ones
        column for the softmax denominator) via PE projections from x^T
  S^T   = K @ Q^T per (head, key-tile) into fp32 PSUM (fp32r operands)
  U     = exp(0.125 * S^T) on ACT (PSUM -> SBUF bf16), masked by (1-mask)^T
        via one DVE multiply
  headsT + denominator via PE (U moving, [V | 1] stationary, bf16)
  normalize via reciprocal + GPSIMD partition-broadcast + DVE multiply,
  output projection (fp32r) accumulated over heads in PSUM.
"""

import sys

sys.path.insert(0, "/opt/trn_rl_repo")

import ml_dtypes
import numpy as np

import concourse.bass as bass  # noqa: F401
import concourse.tile as tile
from concourse import bacc, mybir
from concourse.bass_utils import run_bass_kernel_spmd

F32 = mybir.dt.float32
F32R = mybir.dt.float32r
BF16 = mybir.dt.bfloat16
F16 = mybir.dt.float16
EXP = mybir.ActivationFunctionType.Exp

B, N, D, H, DK = 4, 2048, 512, 8, 64
NQ = N // 2          # query rows per core
NORM = 1.0 / 8.0     # 1/sqrt(DK)
NFC = D // 128       # feature chunks (4)
NHP = H // 2         # head pairs (4)
NGT = N // 128       # key tiles (16)
NQT = NQ // 128      # query tiles per core (8)
NCORES = 8

_CACHE = {}


def _build():
    if "nc" in _CACHE:
        return _CACHE["nc"]
    nc = bacc.Bacc("TRN2", target_bir_lowering=False, debug=False,
                   num_devices=NCORES)
    xqt = nc.dram_tensor("xqt", [D, N], F32, kind="ExternalInput")
    nmtd = nc.dram_tensor("nmt", [N, NQ], F16, kind="ExternalInput")
    wq = nc.dram_tensor("wq", [D, D], F32, kind="ExternalInput")
    wk = nc.dram_tensor("wk", [D, D], F32, kind="ExternalInput")
    wv = nc.dram_tensor("wv", [D, D], F32, kind="ExternalInput")
    wo = nc.dram_tensor("wo", [DK, H * D], F32, kind="ExternalInput")
    out = nc.dram_tensor("out", [NQ, D], F32, kind="ExternalOutput")

    with tile.TileContext(nc) as tc:
        with tc.tile_pool(name="persist", bufs=1) as P:
            kt = P.tile([128, NHP, N], F32R)    # K^T two heads per tile
            qt_ = P.tile([128, NHP, NQ], F32R)  # Q^T two heads per tile
            v_ = P.tile([128, NGT, H, DK + 1], F16)  # V | ones
            nmt = P.tile([128, NGT, NQ], F16)        # (1-mask)^T
            nc.vector.memset(v_[:, :, :, DK:DK + 1], 1.0)
            nbias = P.tile([128, 1], F32)
            nc.vector.memset(nbias[:], -8.0)
            nc.sync.dma_start(
                out=nmt[:],
                in_=nmtd.rearrange("(gc p) q -> p gc q", p=128))

            # ---------------- phase A: loads + projections ----------------
            with tc.tile_pool(name="paps", bufs=3, space="PSUM") as APs, \
                 tc.tile_pool(name="xtp", bufs=1) as XT:
                xt = XT.tile([128, NFC, N], F32R)
                wqb = XT.tile([128, NFC, D], F32R)
                wkb = XT.tile([128, NFC, D], F32R)
                wvb = XT.tile([128, NFC, D], F32R)
                for fc in range(NFC):
                    nc.sync.dma_start(
                        out=xt[:, fc, :],
                        in_=xqt[fc * 128:(fc + 1) * 128, :].bitcast(F32R))
                    for dram, dst in ((wq, wqb), (wk, wkb), (wv, wvb)):
                        nc.sync.dma_start(
                            out=dst[:, fc, :],
                            in_=dram[fc * 128:(fc + 1) * 128, :].bitcast(F32R))

                for hp in range(NHP):
                    for ttg in range(4):
                        ps = APs.tile([128, 512], F32, tag="projps")
                        for fc in range(NFC):
                            nc.tensor.matmul(
                                ps[:],
                                wkb[:, fc, hp * 128:(hp + 1) * 128],
                                xt[:, fc, ttg * 512:(ttg + 1) * 512],
                                start=(fc == 0), stop=(fc == NFC - 1))
                        nc.vector.tensor_copy(
                            kt[:, hp, ttg * 512:(ttg + 1) * 512], ps[:])
                    for ttg in range(2):
                        ps = APs.tile([128, 512], F32, tag="projps")
                        for fc in range(NFC):
                            nc.tensor.matmul(
                                ps[:],
                                wqb[:, fc, hp * 128:(hp + 1) * 128],
                                xt[:, fc, ttg * 512:(ttg + 1) * 512],
                                start=(fc == 0), stop=(fc == NFC - 1))
                        nc.vector.tensor_copy(
                            qt_[:, hp, ttg * 512:(ttg + 1) * 512], ps[:])
                for gt in range(NGT):
                    ps = APs.tile([128, 512], F32, tag="projps")
                    for fc in range(NFC):
                        nc.tensor.matmul(
                            ps[:],
                            xt[:, fc, gt * 128:(gt + 1) * 128],
                            wvb[:, fc, :],
                            start=(fc == 0), stop=(fc == NFC - 1))
                    nc.vector.tensor_copy(
                        v_[:, gt, :, 0:DK],
                        ps.rearrange("p (h v) -> p h v", h=H))

            # rows 0..63 headsT, row 64 denominator; dsum = recip-ready rows
            with tc.tile_pool(name="late", bufs=1) as L:
                hts = L.tile([DK + 1, H, NQ], F32)
                dsum = L.tile([1, H, NQ], F32)
                wob = L.tile([DK, H * D], F32R)
                nc.sync.dma_start(out=wob[:], in_=wo[:, :].bitcast(F32R))

                # ---------------- phase B ----------------
                with tc.tile_pool(name="ub", bufs=3) as UB, \
                     tc.tile_pool(name="spsp", bufs=2, space="PSUM") as SPs, \
                     tc.tile_pool(name="hvp", bufs=1, space="PSUM") as HVs:
                    for hp in range(NHP):
                        hv = [HVs.tile([DK + 1, NQ], F32, tag=f"hv{i}",
                                       name=f"hv{i}") for i in range(2)]
                        for gt in range(NGT):
                            for i in range(2):
                                h = hp * 2 + i
                                s = SPs.tile([128, NQ], F32, tag="sps")
                                for qg in range(2):
                                    nc.tensor.matmul(
                                        s[:, qg * 512:(qg + 1) * 512],
                                        kt[i * 64:(i + 1) * 64, hp,
                                           gt * 128:(gt + 1) * 128],
                                        qt_[i * 64:(i + 1) * 64, hp,
                                            qg * 512:(qg + 1) * 512],
                                        start=True, stop=True)
                                u = UB.tile([128, NQ], F16, tag="u")
                                nc.scalar.activation(u[:], s[:], EXP,
                                                     bias=nbias[:],
                                                     scale=NORM)
                                nc.vector.tensor_mul(u[:], u[:], nmt[:, gt, :])
                                for qg in range(2):
                                    nc.tensor.matmul(
                                        hv[i][:, qg * 512:(qg + 1) * 512],
                                        v_[:, gt, h, :],
                                        u[:, qg * 512:(qg + 1) * 512],
                                        start=(gt == 0), stop=(gt == NGT - 1))
                        for i in range(2):
                            h = hp * 2 + i
                            nc.vector.tensor_copy(hts[:, h, :], hv[i][:])
                            nc.scalar.copy(dsum[0:1, h, :],
                                           hts[DK:DK + 1, h, :])

                # ---------------- phase C/D (two query-half passes) --------
                nc.vector.reciprocal(dsum[:], dsum[:])
                with tc.tile_pool(name="pd", bufs=2) as DP, \
                     tc.tile_pool(name="pdps", bufs=2, space="PSUM") as DPs, \
                     tc.tile_pool(name="htnp", bufs=1) as HTN:
                    HQ = NQ // 2
                    for half in range(2):
                        htn = HTN.tile([DK, H, HQ], F32R, tag="htn")
                        for h in range(H):
                            rinvb = DP.tile([DK, HQ], F32, tag="rinvb")
                            nc.gpsimd.partition_broadcast(
                                rinvb[:],
                                dsum[0:1, h, half * HQ:(half + 1) * HQ])
                            nc.vector.tensor_mul(
                                htn[:, h, :],
                                hts[0:DK, h, half * HQ:(half + 1) * HQ],
                                rinvb[:])
                        for qq in range(HQ // 128):
                            qt = half * (HQ // 128) + qq
                            po = DPs.tile([128, 512], F32, tag="po")
                            for h in range(H):
                                nc.tensor.matmul(
                                    po[:],
                                    htn[:, h, qq * 128:(qq + 1) * 128],
                                    wob[:, h * D:(h + 1) * D],
                                    start=(h == 0), stop=(h == H - 1))
                            ob = DP.tile([128, 512], F32, tag="ob")
                            nc.vector.tensor_copy(ob[:], po[:])
                            nc.sync.dma_start(
                                out=out[qt * 128:(qt + 1) * 128, :], in_=ob[:])

    nc.compile()
    _CACHE["nc"] = nc
    return nc


def kernel(q, mask, W_query, W_key, W_val, W_out):
    q = np.asarray(q, dtype=np.float32)
    mask = np.asarray(mask, dtype=np.int32)
    # [f, h*64+k] layouts for the projections, [k, h*512+e] for the output
    wq_r = np.ascontiguousarray(
        np.transpose(np.asarray(W_query, np.float32), (1, 0, 2)).reshape(D, D))
    wk_r = np.ascontiguousarray(
        np.transpose(np.asarray(W_key, np.float32), (1, 0, 2)).reshape(D, D))
    wv_r = np.ascontiguousarray(
        np.transpose(np.asarray(W_val, np.float32), (1, 0, 2)).reshape(D, D))
    wo_r = np.ascontiguousarray(
        np.transpose(np.asarray(W_out, np.float32), (1, 0, 2)).reshape(DK, H * D))

    nc = _build()
    in_maps = []
    for c in range(NCORES):
        b, qh = c // 2, c % 2
        xqt_c = q[b].T                                      # (D, N)
        nmt_c = 1.0 - mask[b, qh * NQ:(qh + 1) * NQ, :].T   # (N, NQ)
        if qh:
            # rotate the key axis so this core's queries are tokens [0, NQ)
            xqt_c = np.roll(xqt_c, -NQ, axis=1)
            nmt_c = np.roll(nmt_c, -NQ, axis=0)
        in_maps.append({
            "xqt": np.ascontiguousarray(xqt_c),
            "nmt": np.ascontiguousarray(nmt_c.astype(np.float16)),
            "wq": wq_r, "wk": wk_r, "wv": wv_r, "wo": wo_r,
        })
    res = run_bass_kernel_spmd(nc, in_maps, core_ids=list(range(NCORES)))
    output = np.empty((B, N, D), np.float32)
    for c in range(NCORES):
        b, qh = c // 2, c % 2
        output[b, qh * NQ:(qh + 1) * NQ, :] = res.results[c]["out"]
    return output
